# revision 13
# baseline (speedup 1.0000x reference)
"""Trainium2 Bass kernel for the thin-plate-spline RBF layer.

reference:  out[b,n,d] = sum_m phi(|x_bn - c_bm|) * w[b,m,d],
            phi(r) = r^2 * log(r + 1e-6)

Device algorithm (per core, N sharded 8 ways):
  t[m,n] = dist2 + delta  via rank-15 bf16 split-precision matmul
      (coordinates centered, split into bf16 hi/lo; bf16 products are
      exact under fp32 PSUM accumulation; delta folded into the |c|^2
      low split row).  Four batches ride the PE as 32-row strips.
  L[m,n] = ln(t)  split across two engines:
      ScalarE Ln (exact), and VectorE fast-log
      (bitcast fp32->int32, one tensor_scalar: i*ln2/2^23 + const,
      max abs err ~0.03 -- well inside the 2e-2 rel tolerance).
      L stored fp16.
  S[(b,k,d),n] = sum_m (0.5*a_k*w_d)[m] L[m,n]  as fp16 matmuls with
      4-way PE column tiling: batch b owns array columns 32b..32b+31
      (15 used), so the four batches' chains stream concurrently
      through separate XBUSes.
  z = S * bcs (VectorE), out = R^T z (PE, column-strip grouped so four
      n-tiles share one PSUM bank), ScalarE copies PSUM->SBUF fp16,
      DMA out.
"""
import sys

sys.path.insert(0, "/opt/trn_rl_repo")

import numpy as np
import ml_dtypes

BF16 = np.dtype(ml_dtypes.bfloat16)
F16 = np.float16

B, M, N, NCORES = 4, 256, 32768, 8
NS = N // NCORES          # 4096 dense points per core
NT = 512                  # n-tile (one PSUM bank of fp32)
NTILES = NS // NT         # 8
HALVES = M // 128         # 2
KD = 15                   # dist2 split-precision rank
DELTA = 5e-5

LN2 = float(np.log(2.0))
FL_GAMMA = 0.24021347485262412       # quadratic fast-log coefficient
FL_S1G = LN2 / (1 << 23) / FL_GAMMA  # scale on the int view
# The d2 matmul inputs are pre-scaled so PSUM holds t' = t * 2^-ESHIFT;
# this keeps the int-view linear term small enough for fp16 Lraw.
ESHIFT = 111
FL_D0 = (127.0 - ESHIFT) * LN2 + 2.0 * FL_GAMMA
# true ln(t) = FL_GAMMA * Lraw - FL_D0,
#   Lraw = i'*FL_S1G - u,  u = (m-3)*m,  m = bitcast((i'&0x7FFFFF)|0x3F800000)
# max abs err ~0.006 (quadratic fit 0.0053 + fp16 rounding).

# ln-tile engine assignment: 32 tiles indexed (nt, h, i); True -> DVE
# quadratic fast-log (3 ops), False -> ScalarE exact Ln.
N_DVE = 6
DVE_TILE = [(idx * N_DVE) % 32 < N_DVE for idx in range(32)]

_compiled = None


def _build_nc():
    import concourse.bacc as bacc
    import concourse.mybir as mybir
    from concourse.tile import TileContext

    f32 = mybir.dt.float32
    f16 = mybir.dt.float16
    i32 = mybir.dt.int32
    bf = mybir.dt.bfloat16
    nc = bacc.Bacc("TRN2")

    daug_d = nc.dram_tensor("daug", [128, NS], bf, kind="ExternalInput")
    bcs_d = nc.dram_tensor("bcs", [128, NS], f16, kind="ExternalInput")
    cpa_d = nc.dram_tensor("cpa", [128, HALVES * 128], bf, kind="ExternalInput")
    wps_d = nc.dram_tensor("wps", [128, 8 * 32], f16, kind="ExternalInput")
    wpsg_d = nc.dram_tensor("wpsg", [128, 8 * 32], f16, kind="ExternalInput")
    rmat_d = nc.dram_tensor("rmat", [128, 32], f16, kind="ExternalInput")
    vvec_d = nc.dram_tensor("vvec", [128, NTILES], f32, kind="ExternalInput")
    out_d = nc.dram_tensor("outb", [128, NTILES * 128], f16, kind="ExternalOutput")

    mult = mybir.AluOpType.mult
    add = mybir.AluOpType.add
    sub = mybir.AluOpType.subtract
    band = mybir.AluOpType.bitwise_and
    bor = mybir.AluOpType.bitwise_or
    Ln = mybir.ActivationFunctionType.Ln
    Copy = mybir.ActivationFunctionType.Copy

    with TileContext(nc) as tc:
        with (
            tc.tile_pool(name="singles", bufs=1) as singles,
            tc.tile_pool(name="lpool", bufs=10) as lpool,
            tc.tile_pool(name="zpool", bufs=3) as zpool,
            tc.tile_pool(name="d2pool", bufs=2, space="PSUM") as d2pool,
            tc.tile_pool(name="spool", bufs=2, space="PSUM") as spool,
            tc.tile_pool(name="opool", bufs=2, space="PSUM") as opool,
        ):
            scratch = singles.tile([128, NT], bf)
            nc.vector.memset(scratch[:], 0.0)
            junk = singles.tile([128, 64], f16)

            # inputs across DGE paths; most-urgent first
            cpa_t = singles.tile([128, HALVES * 128], bf)
            nc.gpsimd.dma_start(out=cpa_t[:], in_=cpa_d[:])
            daug_t = singles.tile([128, NS], bf)
            for c in range(4):
                csl = slice(c * (NS // 4), (c + 1) * (NS // 4))
                nc.sync.dma_start(out=daug_t[:, csl], in_=daug_d[:, csl])
            wps_t = singles.tile([128, 8 * 32], f16)
            nc.gpsimd.dma_start(out=wps_t[:], in_=wps_d[:])
            wpsg_t = singles.tile([128, 8 * 32], f16)
            nc.gpsimd.dma_start(out=wpsg_t[:], in_=wpsg_d[:])
            rmat_t = singles.tile([128, 32], f16)
            nc.gpsimd.dma_start(out=rmat_t[:], in_=rmat_d[:])
            vvec_t = singles.tile([128, NTILES], f32)
            nc.gpsimd.dma_start(out=vvec_t[:], in_=vvec_d[:])
            bcs_t = singles.tile([128, NS], f16)
            nc.gpsimd.dma_start(out=bcs_t[:, : NS // 2], in_=bcs_d[:, : NS // 2])
            nc.gpsimd.dma_start(out=bcs_t[:, NS // 2 :], in_=bcs_d[:, NS // 2 :])
            out_sb = singles.tile([128, NTILES * 128], f16)

            # HAM warmup on junk data + ACT table preload while DMAs land
            wtile = d2pool.tile([128, 2 * NT], f32, tag="d2")
            for _ in range(2):
                nc.tensor.matmul(
                    wtile[:, : NT // 2], scratch[:, :128], scratch[:, : NT // 2],
                    start=True, stop=True,
                )
            nc.scalar.activation(out=junk[:], in_=scratch[:, :64], func=Ln,
                                 bias=1.0, scale=1.0)

            tidx = 0
            ltiles = {}
            o2 = None
            for nt_ in range(NTILES):
                nsl = slice(nt_ * NT, (nt_ + 1) * NT)
                # ---- dist2 + ln ----
                for h in range(HALVES):
                    for i in range(2):          # batch pair {2i, 2i+1}
                        d2 = d2pool.tile([128, 2 * NT], f32, tag="d2")
                        for bi in range(2):
                            b = 2 * i + bi
                            nc.tensor.matmul(
                                d2[:, bi * NT : (bi + 1) * NT],
                                cpa_t[32 * b : 32 * b + KD,
                                      h * 128 : (h + 1) * 128],
                                daug_t[32 * b : 32 * b + KD, nsl],
                                start=True,
                                stop=True,
                                tile_position=(32 * b, 0),
                            )
                        if DVE_TILE[tidx]:
                            mi = lpool.tile([128, 2 * NT], i32, tag="mi",
                                            bufs=2)
                            nc.vector.tensor_scalar(
                                out=mi[:], in0=d2[:].bitcast(i32),
                                scalar1=0x007FFFFF, scalar2=0x3F800000,
                                op0=band, op1=bor,
                            )
                            ut = lpool.tile([128, 2 * NT], f16, tag="u",
                                            bufs=2)
                            nc.vector.scalar_tensor_tensor(
                                out=ut[:], in0=mi[:].bitcast(f32),
                                scalar=-3.0, in1=mi[:].bitcast(f32),
                                op0=add, op1=mult,
                            )
                            lt = lpool.tile([128, 2 * NT], f16, tag="L")
                            nc.vector.scalar_tensor_tensor(
                                out=lt[:], in0=d2[:].bitcast(i32),
                                scalar=FL_S1G, in1=ut[:],
                                op0=mult, op1=sub,
                            )
                            ltiles[(h, i)] = (lt, True)
                        else:
                            lt = lpool.tile([128, 2 * NT], f16, tag="L")
                            nc.scalar.activation(out=lt[:], in_=d2[:],
                                                 func=Ln,
                                                 scale=float(2.0 ** ESHIFT))
                            ltiles[(h, i)] = (lt, False)
                        tidx += 1

                # ---- S chains: 4-way column-tiled fp16 matmuls ----
                s_c = spool.tile([128, NT], f32, tag="S")
                for b in range(4):
                    for h in range(HALVES):
                        lt, is_dve = ltiles[(h, b // 2)]
                        col = (b % 2) * NT
                        wsrc = wpsg_t if is_dve else wps_t
                        nc.tensor.matmul(
                            s_c[32 * b : 32 * b + 32, :],
                            wsrc[:, 64 * b + 32 * h : 64 * b + 32 * h + 32],
                            lt[:, col : col + NT],
                            start=(h == 0),
                            stop=(h == 1),
                            tile_position=(0, 32 * b),
                        )

                # ---- z = (S + v) * bcs, out = R^T z ----
                z_t = zpool.tile([128, NT], f16, tag="z")
                nc.vector.scalar_tensor_tensor(
                    out=z_t[:], in0=s_c[:], scalar=vvec_t[:, nt_ : nt_ + 1],
                    in1=bcs_t[:, nsl], op0=add, op1=mult,
                )
                q = nt_ % 4
                if q == 0:
                    o2 = opool.tile([128, NT], f32, tag="o2")
                nc.tensor.matmul(o2[32 * q : 32 * q + 32, :],
                                 rmat_t[:], z_t[:],
                                 start=True, stop=True,
                                 tile_position=(0, 32 * q))
                if q == 3:
                    p = nt_ // 4
                    osl = slice(p * NT, (p + 1) * NT)
                    nc.scalar.activation(out=out_sb[:, osl], in_=o2[:],
                                         func=Copy, scale=1.0)
                    nc.sync.dma_start(out=out_d[:, osl], in_=out_sb[:, osl])

    nc.compile()
    return nc


def _split3(v):
    """3-way bf16 split of float64 array."""
    hi = v.astype(BF16)
    r1 = v - hi.astype(np.float64)
    mid = r1.astype(BF16)
    r2 = r1 - mid.astype(np.float64)
    lo = r2.astype(BF16)
    return hi, mid, lo


def _host_prep(sparse_disp, original_cp, original_dense):
    """Build per-core input maps for the device kernel."""
    x = original_dense.astype(np.float64) - 0.5   # (B, N, 3) centered
    c = original_cp.astype(np.float64) - 0.5      # (B, M, 3)
    w = sparse_disp.astype(np.float32)            # (B, M, 3)

    # ---- control-point side (shared by all cores) ----
    p = c.astype(BF16)
    q = (c - p.astype(np.float64)).astype(BF16)
    t_hi, t_mid, t_lo = _split3((c * c).sum(-1))
    # fold the ln bias delta into the low split (values ~5e-5, exact in bf16)
    t_lo = (t_lo.astype(np.float64) + DELTA).astype(BF16)
    ones_m = np.ones((B, M), BF16)

    cpa_full = np.empty((B, KD, M), BF16)
    for d in range(3):
        cpa_full[:, d, :] = p[:, :, d]
        cpa_full[:, 3 + d, :] = p[:, :, d]
        cpa_full[:, 6 + d, :] = q[:, :, d]
    cpa_full[:, 9, :] = t_hi
    cpa_full[:, 10, :] = t_mid
    cpa_full[:, 11, :] = t_lo
    cpa_full[:, 12, :] = ones_m
    cpa_full[:, 13, :] = ones_m
    cpa_full[:, 14, :] = ones_m

    cpa = np.zeros((128, HALVES * 128), BF16)
    for b in range(B):
        for h in range(HALVES):
            cpa[32 * b : 32 * b + KD, h * 128 : (h + 1) * 128] = \
                cpa_full[b, :, h * 128 : (h + 1) * 128]
    # exact power-of-2 pre-scale: with daug's 2^-55 this puts
    # t' = t * 2^-ESHIFT in PSUM
    cpa = (cpa.astype(np.float32) * (2.0 ** -(ESHIFT - 55))).astype(BF16)

    # stationary S-weights: block (b,h) at cols 64b+32h, width 32 (15 used)
    wps32 = np.zeros((128, 8 * 32), np.float32)
    c32 = c.astype(np.float32)
    a5 = np.stack(
        [c32[:, :, 0], c32[:, :, 1], c32[:, :, 2],
         (c32 * c32).sum(-1), np.ones((B, M), np.float32)],
        axis=1,
    )  # (B, 5, M)
    for b in range(B):
        for h in range(HALVES):
            msl = slice(h * 128, (h + 1) * 128)
            blk = np.zeros((128, 32), np.float32)
            for k in range(5):
                for d in range(3):
                    blk[:, k * 3 + d] = 0.5 * a5[b, k, msl] * w[b, msl, d]
            wps32[:, 64 * b + 32 * h : 64 * b + 32 * h + 32] = blk
    wps = wps32.astype(F16)
    wpsg = (FL_GAMMA * wps32).astype(F16)

    # per-n-tile correction vector: true L = GAMMA*Lraw - D0 for DVE
    # tiles.  The chain actually multiplies the f16-quantized wpsg, so
    # use its column sums for exact constant cancellation:
    # v[j] = -(D0/GAMMA) * sum_{m in DVE halves} wpsg_f16[m, j]
    vvec = np.zeros((128, NTILES), np.float32)
    colsum_g = wpsg.astype(np.float32).sum(axis=0) * (FL_D0 / FL_GAMMA)
    for nt_ in range(NTILES):
        for b in range(B):
            for h in range(HALVES):
                tidx = nt_ * 4 + h * 2 + (b // 2)
                if DVE_TILE[tidx]:
                    for kd in range(15):
                        j = 32 * b + kd
                        vvec[j, nt_] += -colsum_g[64 * b + 32 * h + kd]

    rmat = np.zeros((128, 32), F16)
    for b in range(B):
        for k in range(5):
            for d in range(3):
                rmat[32 * b + k * 3 + d, 3 * b + d] = 1.0

    # ---- dense-point side (per core) ----
    u_all = x.astype(BF16)
    v_all = (x - u_all.astype(np.float64)).astype(BF16)
    s_all = (x * x).sum(-1)

    in_maps = []
    for core in range(NCORES):
        csl = slice(core * NS, (core + 1) * NS)
        u = u_all[:, csl, :].astype(np.float32)
        v = v_all[:, csl, :].astype(np.float32)
        s_hi, s_mid, s_lo = _split3(s_all[:, csl])
        ones_n = np.ones((B, NS), BF16)

        daug_b = np.empty((B, KD, NS), BF16)
        for d in range(3):
            daug_b[:, d, :] = (-2.0 * u[:, :, d]).astype(BF16)
            daug_b[:, 3 + d, :] = (-2.0 * v[:, :, d]).astype(BF16)
            daug_b[:, 6 + d, :] = (-2.0 * u[:, :, d]).astype(BF16)
        daug_b[:, 9, :] = ones_n
        daug_b[:, 10, :] = ones_n
        daug_b[:, 11, :] = ones_n
        daug_b[:, 12, :] = s_hi
        daug_b[:, 13, :] = s_mid
        daug_b[:, 14, :] = s_lo

        daug = np.zeros((128, NS), BF16)
        for b in range(B):
            daug[32 * b : 32 * b + KD] = daug_b[b]
        daug = (daug.astype(np.float32) * (2.0 ** -55)).astype(BF16)

        xs = x[:, csl, :].astype(np.float32)
        baug5 = np.stack(
            [-2.0 * xs[:, :, 0], -2.0 * xs[:, :, 1], -2.0 * xs[:, :, 2],
             np.ones((B, NS), np.float32), (xs * xs).sum(-1)],
            axis=1,
        )  # (B, 5, NS)
        bc = np.zeros((128, NS), F16)
        for b in range(B):
            for k in range(5):
                for d in range(3):
                    bc[32 * b + k * 3 + d] = baug5[b, k].astype(F16)

        in_maps.append(
            {
                "daug": daug,
                "bcs": bc,
                "cpa": cpa,
                "wps": wps,
                "wpsg": wpsg,
                "rmat": rmat,
                "vvec": vvec,
            }
        )
    return in_maps


def _assemble(results):
    out = np.empty((B, N, 3), np.float32)
    for core, r in enumerate(results):
        o = r["outb"].astype(np.float32)  # (128, 1024)
        # row 32q + 3b + d, col p*512 + j  ->  n = core*4096 + (4p+q)*512 + j
        for p in range(2):
            for qq in range(4):
                nt_ = 4 * p + qq
                n0 = core * NS + nt_ * NT
                blk = o[32 * qq : 32 * qq + 12, p * NT : (p + 1) * NT]
                out[:, n0 : n0 + NT, :] = (
                    blk.reshape(B, 3, NT).transpose(0, 2, 1)
                )
    return out


def kernel(sparse_disp, original_cp, original_dense):
    global _compiled
    from concourse.bass_utils import run_bass_kernel_spmd

    if _compiled is None:
        _compiled = _build_nc()
    in_maps = _host_prep(sparse_disp, original_cp, original_dense)
    res = run_bass_kernel_spmd(_compiled, in_maps, core_ids=list(range(NCORES)))
    return _assemble(res.results)


# revision 16
# speedup vs baseline: 1.2481x; 1.2481x over previous
"""Trainium2 Bass kernel for the thin-plate-spline RBF layer.

reference:  out[b,n,d] = sum_m phi(|x_bn - c_bm|) * w[b,m,d],
            phi(r) = r^2 * log(r + 1e-6)

Device algorithm (per core, N sharded 8 ways):
  t[m,n] = dist2 + delta  via rank-15 bf16 split-precision matmul
      (coordinates centered, split into bf16 hi/lo; bf16 products are
      exact under fp32 PSUM accumulation; delta folded into the |c|^2
      low split row).  Four batches ride the PE as 32-row strips.
  L[m,n] = ln(t)  split across two engines:
      ScalarE Ln (exact), and VectorE fast-log
      (bitcast fp32->int32, one tensor_scalar: i*ln2/2^23 + const,
      max abs err ~0.03 -- well inside the 2e-2 rel tolerance).
      L stored fp16.
  S[(b,k,d),n] = sum_m (0.5*a_k*w_d)[m] L[m,n]  as fp16 matmuls with
      4-way PE column tiling: batch b owns array columns 32b..32b+31
      (15 used), so the four batches' chains stream concurrently
      through separate XBUSes.
  z = S * bcs (VectorE), out = R^T z (PE, column-strip grouped so four
      n-tiles share one PSUM bank), ScalarE copies PSUM->SBUF fp16,
      DMA out.
"""
import sys

sys.path.insert(0, "/opt/trn_rl_repo")

import numpy as np
import ml_dtypes

BF16 = np.dtype(ml_dtypes.bfloat16)
F16 = np.float16

B, M, N, NCORES = 4, 256, 32768, 8
NS = N // NCORES          # 4096 dense points per core
NT = 512                  # n-tile (one PSUM bank of fp32)
NTILES = NS // NT         # 8
HALVES = M // 128         # 2
KD = 15                   # dist2 split-precision rank
DELTA = 5e-5

LN2 = float(np.log(2.0))
FL_GAMMA = 0.24021347485262412       # quadratic fast-log coefficient
FL_S1G = LN2 / (1 << 23) / FL_GAMMA  # scale on the int view
# The d2 matmul inputs are pre-scaled so PSUM holds t' = t * 2^-ESHIFT;
# this keeps the int-view linear term small enough for fp16 Lraw.
ESHIFT = 111
FL_D0 = (127.0 - ESHIFT) * LN2 + 2.0 * FL_GAMMA
# true ln(t) = FL_GAMMA * Lraw - FL_D0,
#   Lraw = i'*FL_S1G - u,  u = (m-3)*m,  m = bitcast((i'&0x7FFFFF)|0x3F800000)
# max abs err ~0.006 (quadratic fit 0.0053 + fp16 rounding).

# ln-tile engine assignment: 32 tiles indexed (nt, h, i); True -> DVE
# quadratic fast-log (3 ops), False -> ScalarE exact Ln.
N_DVE = 6
DVE_TILE = [(idx * N_DVE) % 32 < N_DVE for idx in range(32)]

_compiled = None


def _build_nc():
    import concourse.bacc as bacc
    import concourse.mybir as mybir
    from concourse.tile import TileContext

    f32 = mybir.dt.float32
    f16 = mybir.dt.float16
    i32 = mybir.dt.int32
    bf = mybir.dt.bfloat16
    nc = bacc.Bacc("TRN2")

    daug_d = nc.dram_tensor("daug", [128, NS], bf, kind="ExternalInput")
    bcs_d = nc.dram_tensor("bcs", [128, NS], f16, kind="ExternalInput")
    cpa_d = nc.dram_tensor("cpa", [128, HALVES * 128], bf, kind="ExternalInput")
    wps_d = nc.dram_tensor("wps", [128, 8 * 32], f16, kind="ExternalInput")
    wpsg_d = nc.dram_tensor("wpsg", [128, 8 * 32], f16, kind="ExternalInput")
    rmat_d = nc.dram_tensor("rmat", [128, 32], f16, kind="ExternalInput")
    vvec_d = nc.dram_tensor("vvec", [128, NTILES], f32, kind="ExternalInput")
    out_d = nc.dram_tensor("outb", [128, NTILES * 128], f16, kind="ExternalOutput")

    mult = mybir.AluOpType.mult
    add = mybir.AluOpType.add
    sub = mybir.AluOpType.subtract
    band = mybir.AluOpType.bitwise_and
    bor = mybir.AluOpType.bitwise_or
    Ln = mybir.ActivationFunctionType.Ln
    Copy = mybir.ActivationFunctionType.Copy

    with TileContext(nc) as tc:
        with (
            tc.tile_pool(name="singles", bufs=1) as singles,
            tc.tile_pool(name="lpool", bufs=10) as lpool,
            tc.tile_pool(name="zpool", bufs=3) as zpool,
            tc.tile_pool(name="d2pool", bufs=3, space="PSUM") as d2pool,
            tc.tile_pool(name="spool", bufs=1, space="PSUM") as spool,
            tc.tile_pool(name="opool", bufs=1, space="PSUM") as opool,
        ):
            scratch = singles.tile([128, NT], bf)
            nc.vector.memset(scratch[:], 0.0)
            junk = singles.tile([128, 64], f16)

            # inputs across DGE paths; most-urgent first
            cpa_t = singles.tile([128, HALVES * 128], bf)
            nc.gpsimd.dma_start(out=cpa_t[:], in_=cpa_d[:])
            daug_t = singles.tile([128, NS], bf)
            _qs = [nc.sync, nc.scalar, nc.sync, nc.scalar]
            for c in range(4):
                csl = slice(c * (NS // 4), (c + 1) * (NS // 4))
                _qs[c].dma_start(out=daug_t[:, csl], in_=daug_d[:, csl])
            wps_t = singles.tile([128, 8 * 32], f16)
            nc.gpsimd.dma_start(out=wps_t[:], in_=wps_d[:])
            wpsg_t = singles.tile([128, 8 * 32], f16)
            nc.gpsimd.dma_start(out=wpsg_t[:], in_=wpsg_d[:])
            rmat_t = singles.tile([128, 32], f16)
            nc.gpsimd.dma_start(out=rmat_t[:], in_=rmat_d[:])
            vvec_t = singles.tile([128, NTILES], f32)
            nc.gpsimd.dma_start(out=vvec_t[:], in_=vvec_d[:])
            bcs_t = singles.tile([128, NS], f16)
            nc.gpsimd.dma_start(out=bcs_t[:, : NS // 2], in_=bcs_d[:, : NS // 2])
            nc.gpsimd.dma_start(out=bcs_t[:, NS // 2 :], in_=bcs_d[:, NS // 2 :])
            out_sb = singles.tile([128, NTILES * 128], f16)

            # HAM warmup on junk data + ACT table preload while DMAs land
            wtile = d2pool.tile([128, 2 * NT], f32, tag="d2")
            for _ in range(2):
                nc.tensor.matmul(
                    wtile[:, : NT // 2], scratch[:, :128], scratch[:, : NT // 2],
                    start=True, stop=True,
                )
            nc.scalar.activation(out=junk[:], in_=scratch[:, :64], func=Ln,
                                 bias=1.0, scale=1.0)

            tidx = 0
            ltiles = {}
            o2 = None
            for nt_ in range(NTILES):
                nsl = slice(nt_ * NT, (nt_ + 1) * NT)
                # ---- dist2 + ln ----
                for h in range(HALVES):
                    for i in range(2):          # batch pair {2i, 2i+1}
                        d2 = d2pool.tile([128, 2 * NT], f32, tag="d2")
                        for bi in range(2):
                            b = 2 * i + bi
                            nc.tensor.matmul(
                                d2[:, bi * NT : (bi + 1) * NT],
                                cpa_t[32 * b : 32 * b + KD,
                                      h * 128 : (h + 1) * 128],
                                daug_t[32 * b : 32 * b + KD, nsl],
                                start=True,
                                stop=True,
                                tile_position=(32 * b, 0),
                            )
                        if DVE_TILE[tidx]:
                            mi = lpool.tile([128, 2 * NT], i32, tag="mi",
                                            bufs=2)
                            nc.vector.tensor_scalar(
                                out=mi[:], in0=d2[:].bitcast(i32),
                                scalar1=0x007FFFFF, scalar2=0x3F800000,
                                op0=band, op1=bor,
                            )
                            ut = lpool.tile([128, 2 * NT], f16, tag="u",
                                            bufs=2)
                            nc.vector.scalar_tensor_tensor(
                                out=ut[:], in0=mi[:].bitcast(f32),
                                scalar=-3.0, in1=mi[:].bitcast(f32),
                                op0=add, op1=mult,
                            )
                            lt = lpool.tile([128, 2 * NT], f16, tag="L")
                            nc.vector.scalar_tensor_tensor(
                                out=lt[:], in0=d2[:].bitcast(i32),
                                scalar=FL_S1G, in1=ut[:],
                                op0=mult, op1=sub,
                            )
                            ltiles[(h, i)] = (lt, True)
                        else:
                            lt = lpool.tile([128, 2 * NT], f16, tag="L")
                            nc.scalar.activation(out=lt[:], in_=d2[:],
                                                 func=Ln,
                                                 scale=float(2.0 ** ESHIFT))
                            ltiles[(h, i)] = (lt, False)
                        tidx += 1

                # ---- S chains: 4-way column-tiled fp16 matmuls ----
                s_c = spool.tile([128, NT], f32, tag="S")
                for h in range(HALVES):
                    for b in range(4):
                        lt, is_dve = ltiles[(h, b // 2)]
                        col = (b % 2) * NT
                        wsrc = wpsg_t if is_dve else wps_t
                        nc.tensor.matmul(
                            s_c[32 * b : 32 * b + 32, :],
                            wsrc[:, 64 * b + 32 * h : 64 * b + 32 * h + 32],
                            lt[:, col : col + NT],
                            start=(h == 0),
                            stop=(h == 1),
                            tile_position=(0, 32 * b),
                        )

                # ---- z = (S + v) * bcs, out = R^T z ----
                z_t = zpool.tile([128, NT], f16, tag="z")
                nc.vector.scalar_tensor_tensor(
                    out=z_t[:], in0=s_c[:], scalar=vvec_t[:, nt_ : nt_ + 1],
                    in1=bcs_t[:, nsl], op0=add, op1=mult,
                )
                q = nt_ % 4
                if q == 0:
                    o2 = opool.tile([128, NT], f32, tag="o2")
                nc.tensor.matmul(o2[32 * q : 32 * q + 32, :],
                                 rmat_t[:], z_t[:],
                                 start=True, stop=True,
                                 tile_position=(0, 32 * q))
                if q == 3:
                    p = nt_ // 4
                    osl = slice(p * NT, (p + 1) * NT)
                    nc.scalar.activation(out=out_sb[:, osl], in_=o2[:],
                                         func=Copy, scale=1.0)
                    nc.sync.dma_start(out=out_d[:, osl], in_=out_sb[:, osl])

    nc.compile()
    return nc


def _split3(v):
    """3-way bf16 split of float64 array."""
    hi = v.astype(BF16)
    r1 = v - hi.astype(np.float64)
    mid = r1.astype(BF16)
    r2 = r1 - mid.astype(np.float64)
    lo = r2.astype(BF16)
    return hi, mid, lo


def _host_prep(sparse_disp, original_cp, original_dense):
    """Build per-core input maps for the device kernel."""
    x = original_dense.astype(np.float64) - 0.5   # (B, N, 3) centered
    c = original_cp.astype(np.float64) - 0.5      # (B, M, 3)
    w = sparse_disp.astype(np.float32)            # (B, M, 3)

    # ---- control-point side (shared by all cores) ----
    p = c.astype(BF16)
    q = (c - p.astype(np.float64)).astype(BF16)
    t_hi, t_mid, t_lo = _split3((c * c).sum(-1))
    # fold the ln bias delta into the low split (values ~5e-5, exact in bf16)
    t_lo = (t_lo.astype(np.float64) + DELTA).astype(BF16)
    ones_m = np.ones((B, M), BF16)

    cpa_full = np.empty((B, KD, M), BF16)
    for d in range(3):
        cpa_full[:, d, :] = p[:, :, d]
        cpa_full[:, 3 + d, :] = p[:, :, d]
        cpa_full[:, 6 + d, :] = q[:, :, d]
    cpa_full[:, 9, :] = t_hi
    cpa_full[:, 10, :] = t_mid
    cpa_full[:, 11, :] = t_lo
    cpa_full[:, 12, :] = ones_m
    cpa_full[:, 13, :] = ones_m
    cpa_full[:, 14, :] = ones_m

    cpa = np.zeros((128, HALVES * 128), BF16)
    for b in range(B):
        for h in range(HALVES):
            cpa[32 * b : 32 * b + KD, h * 128 : (h + 1) * 128] = \
                cpa_full[b, :, h * 128 : (h + 1) * 128]
    # exact power-of-2 pre-scale: with daug's 2^-55 this puts
    # t' = t * 2^-ESHIFT in PSUM
    cpa = (cpa.astype(np.float32) * (2.0 ** -(ESHIFT - 55))).astype(BF16)

    # stationary S-weights: block (b,h) at cols 64b+32h, width 32 (15 used)
    wps32 = np.zeros((128, 8 * 32), np.float32)
    c32 = c.astype(np.float32)
    a5 = np.stack(
        [c32[:, :, 0], c32[:, :, 1], c32[:, :, 2],
         (c32 * c32).sum(-1), np.ones((B, M), np.float32)],
        axis=1,
    )  # (B, 5, M)
    for b in range(B):
        for h in range(HALVES):
            msl = slice(h * 128, (h + 1) * 128)
            blk = np.zeros((128, 32), np.float32)
            for k in range(5):
                for d in range(3):
                    blk[:, k * 3 + d] = 0.5 * a5[b, k, msl] * w[b, msl, d]
            wps32[:, 64 * b + 32 * h : 64 * b + 32 * h + 32] = blk
    wps = wps32.astype(F16)
    wpsg = (FL_GAMMA * wps32).astype(F16)

    # per-n-tile correction vector: true L = GAMMA*Lraw - D0 for DVE
    # tiles.  The chain actually multiplies the f16-quantized wpsg, so
    # use its column sums for exact constant cancellation:
    # v[j] = -(D0/GAMMA) * sum_{m in DVE halves} wpsg_f16[m, j]
    vvec = np.zeros((128, NTILES), np.float32)
    colsum_g = wpsg.astype(np.float32).sum(axis=0) * (FL_D0 / FL_GAMMA)
    for nt_ in range(NTILES):
        for b in range(B):
            for h in range(HALVES):
                tidx = nt_ * 4 + h * 2 + (b // 2)
                if DVE_TILE[tidx]:
                    for kd in range(15):
                        j = 32 * b + kd
                        vvec[j, nt_] += -colsum_g[64 * b + 32 * h + kd]

    rmat = np.zeros((128, 32), F16)
    for b in range(B):
        for k in range(5):
            for d in range(3):
                rmat[32 * b + k * 3 + d, 3 * b + d] = 1.0

    # ---- dense-point side (per core) ----
    u_all = x.astype(BF16)
    v_all = (x - u_all.astype(np.float64)).astype(BF16)
    s_all = (x * x).sum(-1)

    in_maps = []
    for core in range(NCORES):
        csl = slice(core * NS, (core + 1) * NS)
        u = u_all[:, csl, :].astype(np.float32)
        v = v_all[:, csl, :].astype(np.float32)
        s_hi, s_mid, s_lo = _split3(s_all[:, csl])
        ones_n = np.ones((B, NS), BF16)

        daug_b = np.empty((B, KD, NS), BF16)
        for d in range(3):
            daug_b[:, d, :] = (-2.0 * u[:, :, d]).astype(BF16)
            daug_b[:, 3 + d, :] = (-2.0 * v[:, :, d]).astype(BF16)
            daug_b[:, 6 + d, :] = (-2.0 * u[:, :, d]).astype(BF16)
        daug_b[:, 9, :] = ones_n
        daug_b[:, 10, :] = ones_n
        daug_b[:, 11, :] = ones_n
        daug_b[:, 12, :] = s_hi
        daug_b[:, 13, :] = s_mid
        daug_b[:, 14, :] = s_lo

        daug = np.zeros((128, NS), BF16)
        for b in range(B):
            daug[32 * b : 32 * b + KD] = daug_b[b]
        daug = (daug.astype(np.float32) * (2.0 ** -55)).astype(BF16)

        xs = x[:, csl, :].astype(np.float32)
        baug5 = np.stack(
            [-2.0 * xs[:, :, 0], -2.0 * xs[:, :, 1], -2.0 * xs[:, :, 2],
             np.ones((B, NS), np.float32), (xs * xs).sum(-1)],
            axis=1,
        )  # (B, 5, NS)
        bc = np.zeros((128, NS), F16)
        for b in range(B):
            for k in range(5):
                for d in range(3):
                    bc[32 * b + k * 3 + d] = baug5[b, k].astype(F16)

        in_maps.append(
            {
                "daug": daug,
                "bcs": bc,
                "cpa": cpa,
                "wps": wps,
                "wpsg": wpsg,
                "rmat": rmat,
                "vvec": vvec,
            }
        )
    return in_maps


def _assemble(results):
    out = np.empty((B, N, 3), np.float32)
    for core, r in enumerate(results):
        o = r["outb"].astype(np.float32)  # (128, 1024)
        # row 32q + 3b + d, col p*512 + j  ->  n = core*4096 + (4p+q)*512 + j
        for p in range(2):
            for qq in range(4):
                nt_ = 4 * p + qq
                n0 = core * NS + nt_ * NT
                blk = o[32 * qq : 32 * qq + 12, p * NT : (p + 1) * NT]
                out[:, n0 : n0 + NT, :] = (
                    blk.reshape(B, 3, NT).transpose(0, 2, 1)
                )
    return out


def kernel(sparse_disp, original_cp, original_dense):
    global _compiled
    from concourse.bass_utils import run_bass_kernel_spmd

    if _compiled is None:
        _compiled = _build_nc()
    in_maps = _host_prep(sparse_disp, original_cp, original_dense)
    res = run_bass_kernel_spmd(_compiled, in_maps, core_ids=list(range(NCORES)))
    return _assemble(res.results)
